# revision 73
# baseline (speedup 1.0000x reference)
"""Trainium2 Bass kernel for nn_AttentionModel (4-layer gated transformer).

Sharding: pure data-parallel over batch (B=16 -> 2 per core, 8 cores, no
collectives). Feature-major activations in bf16 (fp32 PSUM accumulate).

v2 perf structure (vs v1 baseline at ~1.0ms):
- Attention phase interleaved at (b,hp,jp) grain: scores matmuls, G-proj
  chunks, and lag-2 ctx/denominator consumption are woven so the PE never
  drains (p-state stays at max clock; v1 ctx matmuls ran at 0.9ns/row).
- V-proj hoisted before the scores loop (vv tiles ready for ctx), V bias
  folded into bo host-side (bo' = bo + bv@Wo), O bias accumulated into the
  O-proj PSUM via a K=1 ones matmul so the gate fusion
  res = x + (tanh+1)*attP needs no separate bias op.
- FF uses native sigmoid gating (reference form): f = (p1+b1)*sigmoid(pg+bg)
  via one scalar_tensor_tensor reading p1 straight from PSUM.
- Residuals bf16; LN broadcast rows copied PSUM->bf16 SBUF so the
  normalize ops run in DVE 2x/4x modes.
- Activation tables: Exp set covers exp/tanh/identity/copy/square; Sqrt
  and Sigmoid sets swapped in via warm dummies off the critical chain.
"""

import os
import sys

for _p in ("/opt/trn_rl_repo",):
    if os.path.isdir(_p) and _p not in sys.path:
        sys.path.insert(0, _p)

import numpy as np
import ml_dtypes

import concourse.bass as bass
import concourse.mybir as mybir
import concourse.tile as tile
from concourse import bacc
from concourse.bass_utils import run_bass_kernel_spmd

F32 = mybir.dt.float32
F32R = mybir.dt.float32r
BF = mybir.dt.float16          # activation dtype (fp16: 10-bit mantissa)
BFE = mybir.dt.bfloat16        # exp outputs need bf16 range
NPBF = np.float16
NPBFE = ml_dtypes.bfloat16
AF = mybir.ActivationFunctionType
OP = mybir.AluOpType

B, S, FC, FO = 16, 512, 24, 16
D, H, DK, FFD, L = 512, 8, 64, 2048, 4
MAXPOS = 512
EPS = 1e-6

NCORES = 8
BL = B // NCORES          # local batch = 2
R = BL * S                # local tokens = 1024
DT = D // 128             # feature tiles = 4
FT = FFD // 128           # ff tiles = 16
HDK = H * DK

# aw blob column bases (per layer, [128, 10240] bf16)
AW_Q, AW_K, AW_V, AW_O, AW_G = 0, 2048, 4096, 6144, 8192
AW_COLS = 10240
# fw blob column bases ([128, 24576] bf16)
FW_1, FW_G, FW_2 = 0, 8192, 16384
FW_COLS = 24576
# param blob columns (fp32 [128, NP])
PL = 68                   # per-layer stride
# per-layer: bq 0, bk 4, (unused 8), bg 12, l1s 16, l1b 20, l2s 24, l2b 28,
#            bf1 32, bfg 48, bf2 64
HB = L * PL               # head base = 272
# head: cgm_b +0, other_b +4, fb1 +8, fl1s +10, fl1b +12, fb2 +14,
#       fl2s +15, fl2b +16, fw3 +17, fb3 +18 (row 0)
NP = HB + 19

_CACHE = {}


def _build():
    nc = bacc.Bacc("TRN2", target_bir_lowering=False, debug=False,
                   num_devices=NCORES)

    def par(name, shape, dt):
        return nc.declare_dram_parameter(name, list(shape), dt, isOutput=False)

    xin_d = par("xin", [FC, R], BF)
    xo_d = par("xo", [FO, BL], BF)
    cgmW_d = par("cgmW", [FC, D], BF)
    posE_d = par("posE", [128, 4 * 512], BFE)
    aw_d = par("aw", [L, 128, AW_COLS], BF)
    fw_d = par("fw", [L, 128, FW_COLS], BF)
    pb_d = par("pb", [128, NP], F32)
    bor_d = par("bor", [L, D], BF)
    hw1_d = par("hw1", [128, 8 * 256], BF)
    hw2_d = par("hw2", [128, 2 * 128 + 1], BF)
    ow_d = par("ow", [FO, D], BF)
    out_ext = nc.declare_dram_parameter("out", [1, BL], F32, isOutput=True)

    with tile.TileContext(nc) as tc:
        with (
            nc.allow_low_precision(reason="bf16 matmul/activation pipeline"),
            tc.tile_pool(name="P", bufs=1) as P,
            tc.tile_pool(name="Q", bufs=1, space="PSUM") as Q,
        ):
            MM = nc.tensor.matmul
            NLAYERS = int(os.environ.get("KLAYERS", L))
            KPROBE = os.environ.get("KPROBE", "")
            if KPROBE:
                dbg_ext = nc.declare_dram_parameter(
                    "dbg", [128, 1024], F32, isOutput=True)
                dbg_done = [False]

                def probe(name, ap):
                    if name != KPROBE or dbg_done[0]:
                        return
                    dbg_done[0] = True
                    pdim = ap.shape[0]
                    fdim = ap.free_size()
                    dt_ = P.tile([128, 1024], F32, tag="dbgt", bufs=1)
                    nc.vector.memset(dt_, 0.0)
                    nc.vector.tensor_copy(
                        dt_[0:pdim, 0:fdim], ap)
                    nc.sync.dma_start(out=dbg_ext[:, :], in_=dt_)
            else:
                def probe(name, ap):
                    pass

            # ---------------- constants ----------------
            ones_col = P.tile([128, 1], BF, tag="c_oc", bufs=1)
            nc.vector.memset(ones_col, 1.0)
            invD_col = P.tile([128, 1], BF, tag="c_id", bufs=1)
            nc.vector.memset(invD_col, 1.0 / D)
            ones_512f = P.tile([1, 512], F32, tag="ln_r", bufs=1)
            nc.vector.memset(ones_512f, 1.0)
            ones_row_r = P.tile([1, 128], F32R, tag="c_orr", bufs=1)
            nc.vector.tensor_copy(ones_row_r, ones_512f[:, 0:128])
            ones_row_h = P.tile([1, 128], BF, tag="c_orh", bufs=1)
            nc.vector.memset(ones_row_h, 1.0)
            ones_512h = P.tile([1, 512], BF, tag="c_o5h", bufs=1)
            nc.vector.memset(ones_512h, 1.0)
            eps2 = P.tile([2, 1], F32, tag="c_e", bufs=1)
            nc.vector.memset(eps2, EPS)

            # ---------------- persistent loads ----------------
            pb_sb = P.tile([128, NP], F32, tag="pb", bufs=1)
            nc.sync.dma_start(out=pb_sb, in_=pb_d[:, :])
            posE_sb = P.tile([128, 2048], BFE, tag="posE", bufs=1)
            nc.sync.dma_start(out=posE_sb, in_=posE_d[:, :])
            xo_sb = P.tile([FO, BL], BF, tag="xo", bufs=1)
            nc.sync.dma_start(out=xo_sb, in_=xo_d[:, :])
            hw2_sb = P.tile([128, 257], BF, tag="hw2", bufs=1)
            nc.sync.dma_start(out=hw2_sb, in_=hw2_d[:, :])
            def load_bor(l):
                t = P.tile([1, D], BF, tag="bor", bufs=2, name=f"bor{l}")
                nc.sync.dma_start(out=t, in_=bor_d[l].unsqueeze(0))
                return t

            dum0o = P.tile([1, 1], F32, tag="dum0", bufs=1)
            nc.scalar.activation(out=dum0o, in_=pb_sb[0:1, 0:1],
                                 func=AF.Exp)

            def col(c, n=1):
                return pb_sb[:, c:c + n]

            # layer weight pools
            def load_aw(l, chunked=False):
                t = P.tile([128, AW_COLS], BF, tag="aw",
                           bufs=(1 if KPROBE else 2),
                           name=f"aw{l}")
                if chunked:
                    for c0 in range(0, AW_COLS, 2048):
                        nc.sync.dma_start(out=t[:, c0:c0 + 2048],
                                          in_=aw_d[l][:, c0:c0 + 2048])
                else:
                    nc.sync.dma_start(out=t, in_=aw_d[l])
                return t

            def load_fw(l):
                t = P.tile([128, FW_COLS], BF, tag="fw", bufs=1,
                           name=f"fw{l}")
                nc.sync.dma_start(out=t, in_=fw_d[l])
                return t

            # ------------- activation tile allocator -------------
            free_tags = ["bA", "bB", "bC", "bD", "bE"]

            def alloc_act():
                tag = free_tags.pop(0)
                tiles = [P.tile([128, R], BF, tag=tag, bufs=4,
                                name=f"{tag}_{nc.next_id()}")
                         for _ in range(DT)]
                return tiles, tag

            def free_act(tag):
                free_tags.append(tag)

            xtmp, xtmp_tag = alloc_act()
            xin_sb = xtmp[0][0:FC, :]
            nc.sync.dma_start(out=xin_sb, in_=xin_d[:, :])
            cgmW_sb = xtmp[1][0:FC, 0:D]
            nc.sync.dma_start(out=cgmW_sb, in_=cgmW_d[:, :])
            aw_sb = load_aw(0, chunked=True)

            def alloc_res():
                tiles = [P.tile([128, R], BF, tag="rf", bufs=4,
                                name=f"rf_{nc.next_id()}")
                         for _ in range(DT)]
                return tiles

            # persistent token-major V (ones-augmented for denominators)
            vv = []
            for rt in range(8):
                t = P.tile([128, H * (DK + 1)], BF, tag="vv", bufs=8,
                           name=f"vv{rt}")
                v3 = t.rearrange("p (h e) -> p h e", e=DK + 1)
                nc.vector.memset(v3[:, :, DK:DK + 1], 1.0)
                vv.append(t)

            # ---------------- input projection ----------------
            xT, xT_tag = alloc_act()
            for nt in range(DT):
                for rc in range(2):
                    ps = Q.tile([128, 512], F32,
                                tag=("B" if (nt * 2 + rc) % 2 == 0
                                     else "C"), bufs=2,
                                name=f"ip_{nc.next_id()}")
                    MM(ps, cgmW_sb[:, nt * 128:(nt + 1) * 128],
                       xin_sb[:, rc * 512:(rc + 1) * 512],
                       start=True, stop=True)
                    nc.scalar.activation(
                        out=xT[nt][:, rc * 512:(rc + 1) * 512], in_=ps,
                        func=AF.Identity, bias=col(HB + nt))

            free_act(xtmp_tag)

            # ---------------- helpers ----------------
            def proj_v(dst, wbase, bcols, src):
                """dst[nt] = src @ W + b, feature-major (bias on DVE)."""
                for nt in range(DT):
                    for rc in range(2):
                        ps = Q.tile([128, 512], F32,
                                    tag=("B" if (nt * 2 + rc) % 2 == 0
                                         else "C"), bufs=2,
                                    name=f"pj_{nc.next_id()}")
                        for kt in range(DT):
                            MM(ps,
                               aw_sb[:, wbase + kt * 512 + nt * 128:
                                     wbase + kt * 512 + nt * 128 + 128],
                               src[kt][:, rc * 512:(rc + 1) * 512],
                               start=(kt == 0), stop=(kt == DT - 1))
                        o = dst[nt][:, rc * 512:(rc + 1) * 512]
                        nc.vector.tensor_scalar(
                            out=o, in0=ps, scalar1=col(bcols + nt),
                            scalar2=None, op0=OP.add)

            dum_f = P.tile([1, 1], F32, tag="dum", bufs=2)
            nc.vector.memset(dum_f, 0.5)
            dum_o = P.tile([1, 1], F32, tag="dum", bufs=2)

            def warm_table(func, anchor=None):
                src_ap = anchor[0:1, 0:1] if anchor is not None else dum_f
                nc.scalar.activation(out=dum_o, in_=src_ap, func=func)

            def layernorm(res, cs, cb, dst, accs=None):
                """dst = LN(res) over features (partitions). Stats for both
                512-token chunks share one [1,1024] row; bf16 SBUF broadcast
                copies keep the normalize ops in DVE fast modes."""
                s1p = Q.tile([128, 1024], F32, tag="A", bufs=2,
                             name=f"s1_{nc.next_id()}")
                s2p = Q.tile([128, 1024], F32, tag="A", bufs=2,
                             name=f"s2_{nc.next_id()}")
                for rc in range(2):
                    sl = slice(rc * 512, (rc + 1) * 512)
                    for kt in range(DT):
                        MM(s1p[0:1, sl], invD_col, res[kt][:, sl],
                           start=(kt == 0), stop=(kt == DT - 1))
                # mu row to SBUF early (scalar), squares meanwhile (DVE)
                musb = P.tile([1, 1024], BF, tag="ln_mu", bufs=1,
                              name=f"mu_{nc.next_id()}")
                nc.scalar.activation(out=musb, in_=s1p[0:1, :], func=AF.Copy)
                for rc in range(2):
                    sl = slice(rc * 512, (rc + 1) * 512)
                    for kt in range(DT):
                        sq = P.tile([128, 512], BF, tag="scr", bufs=3,
                                    name=f"sq_{nc.next_id()}")
                        nc.vector.tensor_mul(sq, res[kt][:, sl],
                                             res[kt][:, sl])
                        MM(s2p[0:1, sl], invD_col, sq,
                           start=(kt == 0), stop=(kt == DT - 1))
                mu2 = P.tile([1, 1024], BF, tag="ln_t", bufs=2,
                             name=f"m2_{nc.next_id()}")
                nc.vector.tensor_mul(mu2, musb, musb)
                var = P.tile([1, 1024], F32, tag="ln_t", bufs=2,
                             name=f"va_{nc.next_id()}")
                nc.vector.scalar_tensor_tensor(
                    var, s2p[0:1, :], 1.0, mu2, op0=OP.mult, op1=OP.subtract)
                sg = P.tile([1, 1024], BF, tag="ln_s", bufs=1,
                            name=f"sg_{nc.next_id()}")
                nc.scalar.activation(out=sg, in_=var, func=AF.Sqrt,
                                     bias=eps2[0:1, :])
                for rc in range(2):
                    sl = slice(rc * 512, (rc + 1) * 512)
                    mub = Q.tile([128, 512], F32, tag="C", bufs=2,
                                 name=f"mb_{nc.next_id()}")
                    MM(mub, ones_row_h, musb[:, sl], start=True, stop=True)
                    sgb = Q.tile([128, 512], F32, tag="B", bufs=2,
                                 name=f"sb_{nc.next_id()}")
                    MM(sgb, ones_row_h, sg[:, sl], start=True, stop=True)
                    mubs = P.tile([128, 512], BF, tag="ln_b", bufs=2,
                                  name=f"ms_{nc.next_id()}")
                    nc.scalar.activation(out=mubs, in_=mub, func=AF.Copy)
                    rsb2 = P.tile([128, 512], F32, tag="ln_r", bufs=1,
                                  name=f"rb2_{nc.next_id()}")
                    nc.vector.reciprocal_approx_fast(out=rsb2, in_=sgb)
                    for kt in range(DT):
                        t1 = P.tile([128, 512], BF, tag="scr", bufs=3,
                                    name=f"t1_{nc.next_id()}")
                        nc.vector.tensor_tensor(t1, res[kt][:, sl], mubs,
                                                OP.subtract)
                        t2 = P.tile([128, 512], BF, tag="scr", bufs=3,
                                    name=f"t2_{nc.next_id()}")
                        nc.vector.scalar_tensor_tensor(
                            t2, t1, col(cs + kt), rsb2,
                            op0=OP.mult, op1=OP.mult)
                        nc.scalar.activation(
                            out=dst[kt][:, sl], in_=t2, func=AF.Identity,
                            bias=col(cb + kt),
                            accum_out=(accs[kt][:, rc:rc + 1]
                                       if accs is not None else None))

            # ---------------- transformer layers ----------------
            for l in range(NLAYERS):
                AB = l * PL
                fw_sb = load_fw(l)       # lands during attention
                bor_l = load_bor(l)

                probe("xt", xT[0])
                # V token-major first: copies drain on gpsimd during Q/K proj
                for rt in range(8):
                    ps = Q.tile([128, 512], F32,
                                tag=("C" if rt % 2 == 0 else "B"), bufs=2,
                                name=f"v_{nc.next_id()}")
                    for kt in range(DT):
                        MM(ps, xT[kt][:, rt * 128:(rt + 1) * 128],
                           aw_sb[:, AW_V + kt * 512:AW_V + kt * 512 + 512],
                           start=(kt == 0), stop=(kt == DT - 1))
                    v3o = vv[rt].rearrange("p (h e) -> p h e", e=DK + 1)
                    nc.scalar.activation(
                        out=v3o[:, :, 0:DK],
                        in_=ps.rearrange("p (h d) -> p h d", d=DK),
                        func=AF.Copy)
                probe("v", vv[0])

                qT, qT_tag = alloc_act()
                proj_v(qT, AW_Q, AB + 0, xT)
                probe("q", qT[0])
                kT, kT_tag = alloc_act()
                proj_v(kT, AW_K, AB + 4, xT)
                probe("k", kT[0])
                gT, gT_tag = alloc_act()
                ctxT, ctx_tag = alloc_act()

                # ---------- interleaved attention ----------
                # units u = (b, hp, jp); ctx sub-chunks lag 2 units.
                pr_tiles = {}

                def do_scores(b, hp, jp):
                    psAs = []
                    for h01 in range(2):
                        psAs.append(Q.tile(
                            [128, 1024], F32, tag="A", bufs=2,
                            name=f"sc_{nc.next_id()}"))
                    for j2 in range(2):
                        jt = jp * 2 + j2
                        for h01 in range(2):
                            hs = slice(h01 * 64, h01 * 64 + 64)
                            MM(psAs[h01][:, j2 * 512:(j2 + 1) * 512],
                               kT[hp][hs, b * 512 + jt * 128:
                                      b * 512 + jt * 128 + 128],
                               qT[hp][hs, b * 512:(b + 1) * 512],
                               start=True, stop=True)
                    prs = []
                    for h01 in range(2):
                        pr = P.tile([128, 1024], BFE, tag="pr", bufs=8,
                                    name=f"pr_{nc.next_id()}")
                        nc.scalar.activation(out=pr, in_=psAs[h01],
                                             func=AF.Exp)
                        eng = nc.gpsimd if h01 == 1 else nc.vector
                        eng.tensor_mul(
                            pr, pr,
                            posE_sb[:, jp * 1024:(jp + 1) * 1024])
                        probe("pr", pr)
                        prs.append(pr)
                    pr_tiles[(b, hp, jp)] = prs

                def do_gchunk(g):
                    nt, rc = g % DT, g // DT
                    ps = Q.tile([128, 512], F32, tag="C", bufs=2,
                                name=f"g_{nc.next_id()}")
                    for kt in range(DT):
                        MM(ps,
                           aw_sb[:, AW_G + kt * 512 + nt * 128:
                                 AW_G + kt * 512 + nt * 128 + 128],
                           xT[kt][:, rc * 512:(rc + 1) * 512],
                           start=(kt == 0), stop=(kt == DT - 1))
                    nc.scalar.activation(
                        out=gT[nt][:, rc * 512:(rc + 1) * 512], in_=ps,
                        func=AF.Tanh, bias=col(AB + 12 + nt))

                def do_ctx(k):
                    b = k // 8
                    hp = (k % 8) // 2
                    h01 = k % 2
                    h = hp * 2 + h01
                    pc = Q.tile([128, 512], F32, tag="B", bufs=2,
                                name=f"pc_{nc.next_id()}")
                    for jt in range(4):
                        MM(pc[0:DK + 1, :],
                           vv[b * 4 + jt][:, h * (DK + 1):
                                          (h + 1) * (DK + 1)],
                           pr_tiles[(b, hp, jt // 2)][h01]
                           [:, (jt % 2) * 512:(jt % 2) * 512 + 512],
                           start=(jt == 0), stop=(jt == 3))
                    probe("pc", pc[0:DK + 1, :])
                    dcp = P.tile([1, 512], F32R, tag="rden",
                                 bufs=2, name=f"dc_{nc.next_id()}")
                    nc.vector.tensor_copy(dcp, pc[DK:DK + 1, :])
                    dnb = Q.tile([64, 512], F32, tag="C", bufs=2,
                                 name=f"bc_{nc.next_id()}")
                    MM(dnb, ones_row_r[:, 0:64], dcp,
                       start=True, stop=True)
                    pbc = P.tile([64, 512], F32, tag="rden", bufs=2,
                                 name=f"rb_{nc.next_id()}")
                    nc.vector.reciprocal_approx_fast(out=pbc, in_=dnb)
                    nc.vector.tensor_mul(
                        ctxT[hp][h01 * 64:h01 * 64 + 64,
                                 b * 512:(b + 1) * 512],
                        pc[0:DK, :], pbc)

                gmap = {0: 0, 1: 1, 2: 2, 4: 3}
                for u in range(16):
                    b, hp, jp = u // 8, (u % 8) // 2, u % 2
                    do_scores(b, hp, jp)
                    if u in gmap:
                        do_gchunk(gmap[u])
                    if u >= 2:
                        do_ctx(u - 2)
                do_gchunk(4)
                do_ctx(14)
                do_gchunk(5)
                do_ctx(15)
                do_gchunk(6)
                do_gchunk(7)
                free_act(qT_tag)
                free_act(kT_tag)

                probe("ctx", ctxT[0])
                probe("g", gT[0])

                # table switch to Sqrt while O-proj runs on PE
                warm_table(AF.Sqrt)

                # ---------- O projection + gated residual ----------
                # attP psum gets bo' added via K=1 ones matmul, then
                # res = x + (gT + 1) * attP   (0.5 gate factor in Wo/bo')
                res = alloc_res()
                for nt in range(DT):
                    for rc in range(2):
                        sl = slice(rc * 512, (rc + 1) * 512)
                        ps = Q.tile([128, 512], F32,
                                    tag=("B" if (nt * 2 + rc) % 2 == 0
                                         else "C"), bufs=2,
                                    name=f"o_{nc.next_id()}")
                        for kt in range(DT):
                            MM(ps,
                               aw_sb[:, AW_O + kt * 512 + nt * 128:
                                     AW_O + kt * 512 + nt * 128 + 128],
                               ctxT[kt][:, sl],
                               start=(kt == 0), stop=False)
                        MM(ps, bor_l[:, nt * 128:(nt + 1) * 128],
                           ones_512h, start=False, stop=True)
                        tm = P.tile([128, 512], BF, tag="scr", bufs=3,
                                    name=f"tm_{nc.next_id()}")
                        nc.vector.scalar_tensor_tensor(
                            tm, gT[nt][:, sl], 1.0, ps,
                            op0=OP.add, op1=OP.mult)
                        nc.vector.tensor_add(res[nt][:, sl], tm,
                                             xT[nt][:, sl])
                probe("att", res[0])
                free_act(xT_tag)
                free_act(gT_tag)
                free_act(ctx_tag)

                probe("res", res[0])
                x1, x1_tag = alloc_act()
                layernorm(res, AB + 16, AB + 20, x1)
                warm_table(AF.Sigmoid, x1[0])
                probe("x1", x1[0])

                # prefetch next layer's attention weights
                if l + 1 < NLAYERS:
                    aw_next = load_aw(l + 1)

                # ---------------- FF ----------------
                res2 = alloc_res()
                for rc in range(2):
                    sl = slice(rc * 512, (rc + 1) * 512)
                    accA = [Q.tile([128, 1024], F32, tag="A", bufs=2,
                                   name=f"fa_{nc.next_id()}")
                            for _ in range(2)]
                    accs = [accA[0][:, 0:512], accA[0][:, 512:1024],
                            accA[1][:, 0:512], accA[1][:, 512:1024]]
                    for nt in range(FT):
                        pg = Q.tile([128, 512], F32, tag="C", bufs=2,
                                    name=f"pg_{nc.next_id()}")
                        for kt in range(DT):
                            MM(pg,
                               fw_sb[:, FW_G + kt * 2048 + nt * 128:
                                     FW_G + kt * 2048 + nt * 128 + 128],
                               x1[kt][:, sl],
                               start=(kt == 0), stop=(kt == DT - 1))
                        tg = P.tile([128, 512], BF, tag="fsc", bufs=3,
                                    name=f"tg_{nc.next_id()}")
                        nc.scalar.activation(out=tg, in_=pg, func=AF.Sigmoid,
                                             bias=col(AB + 48 + nt))
                        p1 = Q.tile([128, 512], F32, tag="B", bufs=2,
                                    name=f"p1_{nc.next_id()}")
                        for kt in range(DT):
                            MM(p1,
                               fw_sb[:, FW_1 + kt * 2048 + nt * 128:
                                     FW_1 + kt * 2048 + nt * 128 + 128],
                               x1[kt][:, sl],
                               start=(kt == 0), stop=(kt == DT - 1))
                        f = P.tile([128, 512], BF, tag="fsc", bufs=3,
                                   name=f"f_{nc.next_id()}")
                        nc.vector.scalar_tensor_tensor(
                            f, p1, col(AB + 32 + nt), tg,
                            op0=OP.add, op1=OP.mult)
                        for dt_ in range(DT):
                            MM(accs[dt_],
                               fw_sb[:, FW_2 + nt * 512 + dt_ * 128:
                                     FW_2 + nt * 512 + dt_ * 128 + 128],
                               f, start=(nt == 0), stop=(nt == FT - 1))
                    for dt_ in range(DT):
                        nc.vector.scalar_tensor_tensor(
                            res2[dt_][:, sl], accs[dt_], col(AB + 64 + dt_),
                            x1[dt_][:, sl], op0=OP.add, op1=OP.add)
                probe("res2", res2[0])
                free_act(x1_tag)

                warm_table(AF.Sqrt, res2[0])
                xT, xT_tag = alloc_act()
                if l == NLAYERS - 1:
                    xsums = [P.tile([128, BL], F32, tag="hacc", bufs=4,
                                    name=f"xs_{nc.next_id()}")
                             for _ in range(DT)]
                    # head weights + xo-projection are independent of
                    # xsums: DMAs overlap LN2, matmuls fill its chain gap
                    hw1t, hw1_tag = alloc_act()
                    nc.sync.dma_start(out=hw1t[0], in_=hw1_d[:, 0:1024])
                    nc.sync.dma_start(out=hw1t[1], in_=hw1_d[:, 1024:2048])
                    ow_sb = hw1t[2][0:FO, 0:D]
                    nc.sync.dma_start(out=ow_sb, in_=ow_d[:, :])
                    hT_xo = []
                    for nt in range(DT):
                        ps = Q.tile([128, BL], F32, tag="B", bufs=2,
                                    name=f"ho_{nc.next_id()}")
                        MM(ps, ow_sb[:, nt * 128:(nt + 1) * 128], xo_sb,
                           start=True, stop=True)
                        ht = P.tile([128, BL], BF, tag="hT", bufs=8,
                                    name=f"hx_{nc.next_id()}")
                        nc.vector.tensor_scalar(out=ht, in0=ps,
                                                scalar1=col(HB + 4 + nt),
                                                scalar2=None, op0=OP.add)
                        hT_xo.append(ht)
                else:
                    xsums = None
                layernorm(res2, AB + 24, AB + 28, xT, accs=xsums)
                probe("xout", xT[0])
                warm_table(AF.Exp, xT[0])
                if l + 1 < NLAYERS:
                    aw_sb = aw_next

            # ---------------- head ----------------
            hT = []
            for kt in range(DT):
                ht = P.tile([128, BL], BF, tag="hT", bufs=8,
                            name=f"hm_{nc.next_id()}")
                nc.vector.tensor_scalar(out=ht, in0=xsums[kt],
                                        scalar1=1.0 / S,
                                        scalar2=None, op0=OP.mult)
                hT.append(ht)
            hT.extend(hT_xo)

            eps1 = eps2[0:1, :]
            warm_table(AF.Sqrt, hT[0])

            def head_ln_relu(zt, n_tiles, nfeat, cs, cb, outtag):
                s1p = Q.tile([1, BL], F32, tag="B", bufs=2,
                             name=f"hs1_{nc.next_id()}")
                for kt in range(n_tiles):
                    MM(s1p, ones_col, zt[kt], start=(kt == 0),
                       stop=(kt == n_tiles - 1))
                s2p = Q.tile([1, BL], F32, tag="C", bufs=2,
                             name=f"hs2_{nc.next_id()}")
                for kt in range(n_tiles):
                    z2 = P.tile([128, BL], BF, tag="hd2", bufs=4,
                                name=f"z2_{nc.next_id()}")
                    nc.vector.tensor_mul(z2, zt[kt], zt[kt])
                    MM(s2p, ones_col, z2, start=(kt == 0),
                       stop=(kt == n_tiles - 1))
                mu = P.tile([1, BL], F32R, tag="hmu", bufs=4,
                            name=f"hmu_{nc.next_id()}")
                nc.vector.tensor_scalar(out=mu, in0=s1p,
                                        scalar1=1.0 / nfeat,
                                        scalar2=None, op0=OP.mult)
                m2 = P.tile([1, BL], F32, tag="hln", bufs=8,
                            name=f"hm2_{nc.next_id()}")
                nc.vector.tensor_scalar(out=m2, in0=s2p,
                                        scalar1=1.0 / nfeat,
                                        scalar2=None, op0=OP.mult)
                var = P.tile([1, BL], F32, tag="hln", bufs=8,
                             name=f"hva_{nc.next_id()}")
                nc.vector.scalar_tensor_tensor(
                    var, mu, -1.0, mu, op0=OP.mult, op1=OP.mult)
                nc.vector.tensor_add(var, var, m2)
                sq = P.tile([1, BL], F32, tag="hln", bufs=8,
                            name=f"hsq_{nc.next_id()}")
                nc.scalar.activation(out=sq, in_=var, func=AF.Sqrt,
                                     bias=eps1)
                rs = P.tile([1, BL], F32, tag="hmu", bufs=4,
                            name=f"hrs_{nc.next_id()}")
                nc.vector.reciprocal_approx_fast(out=rs, in_=sq)
                rsr = P.tile([1, BL], F32R, tag="hmu", bufs=4,
                             name=f"hrr_{nc.next_id()}")
                nc.vector.tensor_copy(rsr, rs)
                mub = Q.tile([128, BL], F32, tag="C", bufs=2,
                             name=f"hmb_{nc.next_id()}")
                MM(mub, ones_row_r, mu, start=True, stop=True)
                rsb = Q.tile([128, BL], F32, tag="B", bufs=2,
                             name=f"hrb_{nc.next_id()}")
                MM(rsb, ones_row_r, rsr, start=True, stop=True)
                outs = []
                for kt in range(n_tiles):
                    t1 = P.tile([128, BL], F32, tag="hd", bufs=8,
                                name=f"ht1_{nc.next_id()}")
                    nc.vector.tensor_tensor(t1, zt[kt], mub, OP.subtract)
                    t2 = P.tile([128, BL], F32, tag="hd", bufs=8,
                                name=f"ht2_{nc.next_id()}")
                    nc.vector.scalar_tensor_tensor(
                        t2, t1, col(cs + kt), rsb, op0=OP.mult, op1=OP.mult)
                    o = P.tile([128, BL], BF, tag=outtag, bufs=4,
                               name=f"ho_{nc.next_id()}")
                    nc.scalar.activation(out=o, in_=t2, func=AF.Relu,
                                         bias=col(cb + kt))
                    outs.append(o)
                return outs

            # fc1 [1024 -> 256]  (weights in recycled act tiles)
            def hw1_sl(c0, c1):
                t = hw1t[c0 // 1024]
                return t[:, c0 % 1024:c0 % 1024 + (c1 - c0)]

            z1 = []
            for nt in range(2):
                ps = Q.tile([128, BL], F32, tag="B", bufs=2,
                            name=f"f1_{nc.next_id()}")
                for kt in range(8):
                    c0 = kt * 256 + nt * 128
                    MM(ps, hw1_sl(c0, c0 + 128), hT[kt],
                       start=(kt == 0), stop=(kt == 7))
                z = P.tile([128, BL], BF, tag="z1", bufs=2,
                           name=f"z1_{nc.next_id()}")
                nc.vector.tensor_scalar(out=z, in0=ps,
                                        scalar1=col(HB + 8 + nt),
                                        scalar2=None, op0=OP.add)
                z1.append(z)
            h1 = head_ln_relu(z1, 2, 256, HB + 10, HB + 12, "h1")

            # fc2 [256 -> 128]
            ps = Q.tile([128, BL], F32, tag="B", bufs=2,
                        name=f"f2_{nc.next_id()}")
            for kt in range(2):
                MM(ps, hw2_sb[:, kt * 128:(kt + 1) * 128], h1[kt],
                   start=(kt == 0), stop=(kt == 1))
            z2_ = P.tile([128, BL], BF, tag="z2", bufs=2,
                         name=f"z2h_{nc.next_id()}")
            nc.vector.tensor_scalar(out=z2_, in0=ps, scalar1=col(HB + 14),
                                    scalar2=None, op0=OP.add)
            h2 = head_ln_relu([z2_], 1, 128, HB + 15, HB + 16, "h2")

            # fc3 [128 -> 1]
            ps = Q.tile([1, BL], F32, tag="C", bufs=2,
                        name=f"f3_{nc.next_id()}")
            MM(ps, hw2_sb[:, 256:257], h2[0], start=True, stop=True)
            out_sb = P.tile([1, BL], F32, tag="outsb", bufs=1)
            nc.vector.tensor_scalar(out=out_sb, in0=ps,
                                    scalar1=pb_sb[0:1, HB + 18:HB + 19],
                                    scalar2=None, op0=OP.add)
            nc.sync.dma_start(out=out_ext[:, :], in_=out_sb)

    nc.compile()
    return nc


def _tile_w(W):
    """[K*128, Dout] -> [128, K*Dout] bf16 (kt-major blocks)."""
    K = W.shape[0] // 128
    return np.ascontiguousarray(
        W.reshape(K, 128, -1).transpose(1, 0, 2).reshape(128, -1)
    ).astype(NPBF)


def _cols(pb, base, vec):
    """Pack vec[len=128*n] into pb[:, base:base+n] column-major."""
    v = np.asarray(vec, np.float32).reshape(-1, 128).T
    pb[:, base:base + v.shape[1]] = v


def _pack_shared(inputs):
    f32 = np.float32
    g = lambda k: np.asarray(inputs[k], f32)

    aw = np.zeros((L, 128, AW_COLS), NPBF)
    fw = np.zeros((L, 128, FW_COLS), NPBF)
    pb = np.zeros((128, NP), f32)
    bor = np.zeros((L, D), f32)
    Wq, bq = g("Wq"), g("bq")
    Wk, bk = g("Wk"), g("bk")
    Wv, bv = g("Wv"), g("bv")
    Wo, bo = g("Wo"), g("bo")
    Wg, bg = g("Wg"), g("bg")
    Wf1, bf1 = g("Wf1"), g("bf1")
    Wfg, bfg = g("Wfg"), g("bfg")
    Wf2, bf2 = g("Wf2"), g("bf2")
    for l in range(L):
        aw[l][:, AW_Q:AW_K] = _tile_w(Wq[l] * 0.125)
        aw[l][:, AW_K:AW_V] = _tile_w(Wk[l])
        aw[l][:, AW_V:AW_O] = _tile_w(Wv[l])
        # 0.5: sigmoid gate = (tanh(z/2)+1)/2, folded into Wo/bo'
        aw[l][:, AW_O:AW_G] = _tile_w(Wo[l] * 0.5)
        aw[l][:, AW_G:] = _tile_w(Wg[l] * 0.5)
        fw[l][:, FW_1:FW_G] = _tile_w(Wf1[l])
        fw[l][:, FW_G:FW_2] = _tile_w(Wfg[l])
        fw[l][:, FW_2:] = _tile_w(Wf2[l])
        AB = l * PL
        _cols(pb, AB + 0, bq[l] * 0.125)
        _cols(pb, AB + 4, bk[l])
        _cols(pb, AB + 12, bg[l] * 0.5)
        _cols(pb, AB + 16, g("ln1_s")[l])
        _cols(pb, AB + 20, g("ln1_b")[l])
        _cols(pb, AB + 24, g("ln2_s")[l])
        _cols(pb, AB + 28, g("ln2_b")[l])
        _cols(pb, AB + 32, bf1[l])
        _cols(pb, AB + 48, bfg[l])
        _cols(pb, AB + 64, bf2[l])
        # bo' = (bo + bv @ Wo) * 0.5, added into the O-proj PSUM by a
        # K=1 matmul (stationary = bor row, moving = ones)
        bor[l] = (bo[l] + bv[l] @ Wo[l]) * 0.5
    _cols(pb, HB + 0, g("cgm_b"))
    _cols(pb, HB + 4, g("other_b"))
    _cols(pb, HB + 8, g("fb1"))
    _cols(pb, HB + 10, g("fln1_s"))
    _cols(pb, HB + 12, g("fln1_b"))
    pb[:, HB + 14] = g("fb2")
    pb[:, HB + 15] = g("fln2_s")
    pb[:, HB + 16] = g("fln2_b")
    pb[:, HB + 17] = g("fW3")[:, 0]
    pb[0, HB + 18] = g("fb3")[0]

    # posE: exp(pos_bias) in scores-transposed layout
    rbar = g("rel_emb").mean(axis=1)            # [1023]
    posE = np.zeros((128, 2048), f32)
    Jv = np.arange(128)[:, None]
    Iv = np.arange(512)[None, :]
    for jt in range(4):
        idx = 511 - 128 * jt - Jv + Iv
        posE[:, jt * 512:(jt + 1) * 512] = np.exp(rbar[idx])

    return {
        "cgmW": g("cgm_W").astype(NPBF),
        "posE": posE.astype(NPBFE),
        "aw": aw, "fw": fw, "pb": pb, "bor": bor.astype(NPBF),
        "hw1": _tile_w(g("fW1")),
        "hw2": np.concatenate([_tile_w(g("fW2")),
                               g("fW3").astype(NPBF)], axis=1),
        "ow": g("other_W").astype(NPBF),
    }


def _get_nc():
    if "nc" not in _CACHE:
        _CACHE["nc"] = _build()
    return _CACHE["nc"]


def kernel(**inputs):
    shared = _pack_shared(inputs)
    x_cgm = np.asarray(inputs["x_cgm"], np.float32)
    x_other = np.asarray(inputs["x_other"], np.float32)
    in_maps = []
    for c in range(NCORES):
        m = dict(shared)
        xs = x_cgm[c * BL:(c + 1) * BL].reshape(R, FC).T
        m["xin"] = np.ascontiguousarray(xs).astype(NPBF)
        m["xo"] = np.ascontiguousarray(
            x_other[c * BL:(c + 1) * BL].T).astype(NPBF)
        in_maps.append(m)

    nc = _get_nc()
    trace = bool(int(os.environ.get("KTRACE", "0")))
    res = run_bass_kernel_spmd(nc, in_maps, core_ids=list(range(NCORES)),
                               trace=trace)
    _CACHE["last_res"] = res
    out = np.concatenate(
        [res.results[c]["out"].reshape(BL, 1) for c in range(NCORES)], axis=0)
    return out.astype(np.float32)


# revision 74
# speedup vs baseline: 1.2801x; 1.2801x over previous
"""Trainium2 Bass kernel for nn_AttentionModel (4-layer gated transformer).

Sharding: pure data-parallel over batch (B=16 -> 2 per core, 8 cores, no
collectives). Feature-major activations in bf16 (fp32 PSUM accumulate).

v2 perf structure (vs v1 baseline at ~1.0ms):
- Attention phase interleaved at (b,hp,jp) grain: scores matmuls, G-proj
  chunks, and lag-2 ctx/denominator consumption are woven so the PE never
  drains (p-state stays at max clock; v1 ctx matmuls ran at 0.9ns/row).
- V-proj hoisted before the scores loop (vv tiles ready for ctx), V bias
  folded into bo host-side (bo' = bo + bv@Wo), O bias accumulated into the
  O-proj PSUM via a K=1 ones matmul so the gate fusion
  res = x + (tanh+1)*attP needs no separate bias op.
- FF uses native sigmoid gating (reference form): f = (p1+b1)*sigmoid(pg+bg)
  via one scalar_tensor_tensor reading p1 straight from PSUM.
- Residuals bf16; LN broadcast rows copied PSUM->bf16 SBUF so the
  normalize ops run in DVE 2x/4x modes.
- Activation tables: Exp set covers exp/tanh/identity/copy/square; Sqrt
  and Sigmoid sets swapped in via warm dummies off the critical chain.
"""

import os
import sys

for _p in ("/opt/trn_rl_repo",):
    if os.path.isdir(_p) and _p not in sys.path:
        sys.path.insert(0, _p)

import numpy as np
import ml_dtypes

import concourse.bass as bass
import concourse.mybir as mybir
import concourse.tile as tile
from concourse import bacc
from concourse.bass_utils import run_bass_kernel_spmd

F32 = mybir.dt.float32
F32R = mybir.dt.float32r
BF = mybir.dt.float16          # activation dtype (fp16: 10-bit mantissa)
BFE = mybir.dt.bfloat16        # exp outputs need bf16 range
NPBF = np.float16
NPBFE = ml_dtypes.bfloat16
AF = mybir.ActivationFunctionType
OP = mybir.AluOpType

B, S, FC, FO = 16, 512, 24, 16
D, H, DK, FFD, L = 512, 8, 64, 2048, 4
MAXPOS = 512
EPS = 1e-6

NCORES = 8
BL = B // NCORES          # local batch = 2
R = BL * S                # local tokens = 1024
DT = D // 128             # feature tiles = 4
FT = FFD // 128           # ff tiles = 16
HDK = H * DK

# aw blob column bases (per layer, [128, 10240] bf16)
AW_Q, AW_K, AW_V, AW_O, AW_G = 0, 2048, 4096, 6144, 8192
AW_COLS = 10240
# fw blob column bases ([128, 24576] bf16)
FW_1, FW_G, FW_2 = 0, 8192, 16384
FW_COLS = 24576
# param blob columns (fp32 [128, NP])
PL = 68                   # per-layer stride
# per-layer: bq 0, bk 4, (unused 8), bg 12, l1s 16, l1b 20, l2s 24, l2b 28,
#            bf1 32, bfg 48, bf2 64
HB = L * PL               # head base = 272
# head: cgm_b +0, other_b +4, fb1 +8, fl1s +10, fl1b +12, fb2 +14,
#       fl2s +15, fl2b +16, fw3 +17, fb3 +18 (row 0)
NP = HB + 19

_CACHE = {}


def _build():
    nc = bacc.Bacc("TRN2", target_bir_lowering=False, debug=False,
                   num_devices=NCORES)

    def par(name, shape, dt):
        return nc.declare_dram_parameter(name, list(shape), dt, isOutput=False)

    xin_d = par("xin", [FC, R], BF)
    xo_d = par("xo", [FO, BL], BF)
    cgmW_d = par("cgmW", [FC, D], BF)
    posE_d = par("posE", [128, 4 * 512], BFE)
    aw_d = par("aw", [L, 128, AW_COLS], BF)
    fw_d = par("fw", [L, 128, FW_COLS], BF)
    pb_d = par("pb", [128, NP], F32)
    bor_d = par("bor", [L, D], BF)
    hw1_d = par("hw1", [128, 8 * 256], BF)
    hw2_d = par("hw2", [128, 2 * 128 + 1], BF)
    ow_d = par("ow", [FO, D], BF)
    out_ext = nc.declare_dram_parameter("out", [1, BL], F32, isOutput=True)

    with tile.TileContext(nc) as tc:
        with (
            nc.allow_low_precision(reason="bf16 matmul/activation pipeline"),
            tc.tile_pool(name="P", bufs=1) as P,
            tc.tile_pool(name="Q", bufs=1, space="PSUM") as Q,
        ):
            MM = nc.tensor.matmul
            NLAYERS = int(os.environ.get("KLAYERS", L))
            KPROBE = os.environ.get("KPROBE", "")
            if KPROBE:
                dbg_ext = nc.declare_dram_parameter(
                    "dbg", [128, 1024], F32, isOutput=True)
                dbg_done = [False]

                def probe(name, ap):
                    if name != KPROBE or dbg_done[0]:
                        return
                    dbg_done[0] = True
                    pdim = ap.shape[0]
                    fdim = ap.free_size()
                    dt_ = P.tile([128, 1024], F32, tag="dbgt", bufs=1)
                    nc.vector.memset(dt_, 0.0)
                    nc.vector.tensor_copy(
                        dt_[0:pdim, 0:fdim], ap)
                    nc.sync.dma_start(out=dbg_ext[:, :], in_=dt_)
            else:
                def probe(name, ap):
                    pass

            # ---------------- constants ----------------
            ones_col = P.tile([128, 1], BF, tag="c_oc", bufs=1)
            nc.vector.memset(ones_col, 1.0)
            invD_col = P.tile([128, 1], BF, tag="c_id", bufs=1)
            nc.vector.memset(invD_col, 1.0 / D)
            ones_512f = P.tile([1, 512], F32, tag="ln_r", bufs=1)
            nc.vector.memset(ones_512f, 1.0)
            ones_row_r = P.tile([1, 128], F32R, tag="c_orr", bufs=1)
            nc.vector.tensor_copy(ones_row_r, ones_512f[:, 0:128])
            ones_row_h = P.tile([1, 128], BF, tag="c_orh", bufs=1)
            nc.vector.memset(ones_row_h, 1.0)
            ones_512h = P.tile([1, 512], BF, tag="c_o5h", bufs=1)
            nc.vector.memset(ones_512h, 1.0)
            eps2 = P.tile([2, 1], F32, tag="c_e", bufs=1)
            nc.vector.memset(eps2, EPS)

            # ---------------- persistent loads ----------------
            pb_sb = P.tile([128, NP], F32, tag="pb", bufs=1)
            nc.sync.dma_start(out=pb_sb, in_=pb_d[:, :])
            posE_sb = P.tile([128, 2048], BFE, tag="posE", bufs=1)
            nc.sync.dma_start(out=posE_sb, in_=posE_d[:, :])
            xo_sb = P.tile([FO, BL], BF, tag="xo", bufs=1)
            nc.sync.dma_start(out=xo_sb, in_=xo_d[:, :])
            hw2_sb = P.tile([128, 257], BF, tag="hw2", bufs=1)
            nc.sync.dma_start(out=hw2_sb, in_=hw2_d[:, :])
            def load_bor(l):
                t = P.tile([1, D], BF, tag="bor", bufs=2, name=f"bor{l}")
                nc.sync.dma_start(out=t, in_=bor_d[l].unsqueeze(0))
                return t

            dum0o = P.tile([1, 1], F32, tag="dum0", bufs=1)
            nc.scalar.activation(out=dum0o, in_=pb_sb[0:1, 0:1],
                                 func=AF.Exp)

            def col(c, n=1):
                return pb_sb[:, c:c + n]

            # layer weight pools
            def load_aw(l, chunked=False):
                t = P.tile([128, AW_COLS], BF, tag="aw",
                           bufs=(1 if KPROBE else 2),
                           name=f"aw{l}")
                if chunked:
                    for c0 in range(0, AW_COLS, 2048):
                        nc.sync.dma_start(out=t[:, c0:c0 + 2048],
                                          in_=aw_d[l][:, c0:c0 + 2048])
                else:
                    nc.sync.dma_start(out=t, in_=aw_d[l])
                return t

            def load_fw(l):
                t = P.tile([128, FW_COLS], BF, tag="fw", bufs=1,
                           name=f"fw{l}")
                nc.sync.dma_start(out=t, in_=fw_d[l])
                return t

            # ------------- activation tile allocator -------------
            free_tags = ["bA", "bB", "bC", "bD", "bE"]

            def alloc_act():
                tag = free_tags.pop(0)
                tiles = [P.tile([128, R], BF, tag=tag, bufs=4,
                                name=f"{tag}_{nc.next_id()}")
                         for _ in range(DT)]
                return tiles, tag

            def free_act(tag):
                free_tags.append(tag)

            xtmp, xtmp_tag = alloc_act()
            xin_sb = xtmp[0][0:FC, :]
            nc.sync.dma_start(out=xin_sb, in_=xin_d[:, :])
            cgmW_sb = xtmp[1][0:FC, 0:D]
            nc.sync.dma_start(out=cgmW_sb, in_=cgmW_d[:, :])
            aw_sb = load_aw(0, chunked=True)

            def alloc_res():
                tiles = [P.tile([128, R], BF, tag="rf", bufs=4,
                                name=f"rf_{nc.next_id()}")
                         for _ in range(DT)]
                return tiles

            # persistent token-major V (ones-augmented for denominators)
            vv = []
            for rt in range(8):
                t = P.tile([128, H * (DK + 1)], BF, tag="vv", bufs=8,
                           name=f"vv{rt}")
                v3 = t.rearrange("p (h e) -> p h e", e=DK + 1)
                nc.vector.memset(v3[:, :, DK:DK + 1], 1.0)
                vv.append(t)

            # ---------------- input projection ----------------
            xT, xT_tag = alloc_act()
            for nt in range(DT):
                for rc in range(2):
                    ps = Q.tile([128, 512], F32,
                                tag=("B" if (nt * 2 + rc) % 2 == 0
                                     else "C"), bufs=2,
                                name=f"ip_{nc.next_id()}")
                    MM(ps, cgmW_sb[:, nt * 128:(nt + 1) * 128],
                       xin_sb[:, rc * 512:(rc + 1) * 512],
                       start=True, stop=True)
                    nc.scalar.activation(
                        out=xT[nt][:, rc * 512:(rc + 1) * 512], in_=ps,
                        func=AF.Identity, bias=col(HB + nt))

            free_act(xtmp_tag)

            # ---------------- helpers ----------------
            def proj_v(dst, wbase, bcols, src):
                """dst[nt] = src @ W + b, feature-major (bias on DVE)."""
                for nt in range(DT):
                    for rc in range(2):
                        ps = Q.tile([128, 512], F32,
                                    tag=("B" if (nt * 2 + rc) % 2 == 0
                                         else "C"), bufs=2,
                                    name=f"pj_{nc.next_id()}")
                        for kt in range(DT):
                            MM(ps,
                               aw_sb[:, wbase + kt * 512 + nt * 128:
                                     wbase + kt * 512 + nt * 128 + 128],
                               src[kt][:, rc * 512:(rc + 1) * 512],
                               start=(kt == 0), stop=(kt == DT - 1))
                        o = dst[nt][:, rc * 512:(rc + 1) * 512]
                        nc.vector.tensor_scalar(
                            out=o, in0=ps, scalar1=col(bcols + nt),
                            scalar2=None, op0=OP.add)

            dum_f = P.tile([1, 1], F32, tag="dum", bufs=2)
            nc.vector.memset(dum_f, 0.5)
            dum_o = P.tile([1, 1], F32, tag="dum", bufs=2)

            def warm_table(func, anchor=None):
                src_ap = anchor[0:1, 0:1] if anchor is not None else dum_f
                nc.scalar.activation(out=dum_o, in_=src_ap, func=func)

            def layernorm(res, cs, cb, dst, accs=None):
                """dst = LN(res) over features (partitions). Stats for both
                512-token chunks share one [1,1024] row; bf16 SBUF broadcast
                copies keep the normalize ops in DVE fast modes."""
                s1p = Q.tile([128, 1024], F32, tag="A", bufs=2,
                             name=f"s1_{nc.next_id()}")
                s2p = Q.tile([128, 1024], F32, tag="A", bufs=2,
                             name=f"s2_{nc.next_id()}")
                for rc in range(2):
                    sl = slice(rc * 512, (rc + 1) * 512)
                    for kt in range(DT):
                        MM(s1p[0:1, sl], invD_col, res[kt][:, sl],
                           start=(kt == 0), stop=(kt == DT - 1))
                # mu row to SBUF early (scalar), squares meanwhile (DVE)
                musb = P.tile([1, 1024], BF, tag="ln_mu", bufs=1,
                              name=f"mu_{nc.next_id()}")
                nc.scalar.activation(out=musb, in_=s1p[0:1, :], func=AF.Copy)
                for rc in range(2):
                    sl = slice(rc * 512, (rc + 1) * 512)
                    for kt in range(DT):
                        sq = P.tile([128, 512], BF, tag="scr", bufs=3,
                                    name=f"sq_{nc.next_id()}")
                        nc.vector.tensor_mul(sq, res[kt][:, sl],
                                             res[kt][:, sl])
                        MM(s2p[0:1, sl], invD_col, sq,
                           start=(kt == 0), stop=(kt == DT - 1))
                mu2 = P.tile([1, 1024], BF, tag="ln_t", bufs=2,
                             name=f"m2_{nc.next_id()}")
                nc.vector.tensor_mul(mu2, musb, musb)
                var = P.tile([1, 1024], F32, tag="ln_t", bufs=2,
                             name=f"va_{nc.next_id()}")
                nc.vector.scalar_tensor_tensor(
                    var, s2p[0:1, :], 1.0, mu2, op0=OP.mult, op1=OP.subtract)
                sg = P.tile([1, 1024], BF, tag="ln_s", bufs=1,
                            name=f"sg_{nc.next_id()}")
                nc.scalar.activation(out=sg, in_=var, func=AF.Sqrt,
                                     bias=eps2[0:1, :])
                for rc in range(2):
                    sl = slice(rc * 512, (rc + 1) * 512)
                    mub = Q.tile([128, 512], F32, tag="C", bufs=2,
                                 name=f"mb_{nc.next_id()}")
                    MM(mub, ones_row_h, musb[:, sl], start=True, stop=True)
                    sgb = Q.tile([128, 512], F32, tag="B", bufs=2,
                                 name=f"sb_{nc.next_id()}")
                    MM(sgb, ones_row_h, sg[:, sl], start=True, stop=True)
                    mubs = P.tile([128, 512], BF, tag="ln_b", bufs=2,
                                  name=f"ms_{nc.next_id()}")
                    nc.scalar.activation(out=mubs, in_=mub, func=AF.Copy)
                    rsb2 = P.tile([128, 512], F32, tag="ln_r", bufs=1,
                                  name=f"rb2_{nc.next_id()}")
                    nc.vector.reciprocal_approx_fast(out=rsb2, in_=sgb)
                    for kt in range(DT):
                        t1 = P.tile([128, 512], BF, tag="scr", bufs=3,
                                    name=f"t1_{nc.next_id()}")
                        nc.vector.tensor_tensor(t1, res[kt][:, sl], mubs,
                                                OP.subtract)
                        t2 = P.tile([128, 512], BF, tag="scr", bufs=3,
                                    name=f"t2_{nc.next_id()}")
                        nc.vector.scalar_tensor_tensor(
                            t2, t1, col(cs + kt), rsb2,
                            op0=OP.mult, op1=OP.mult)
                        nc.scalar.activation(
                            out=dst[kt][:, sl], in_=t2, func=AF.Identity,
                            bias=col(cb + kt),
                            accum_out=(accs[kt][:, rc:rc + 1]
                                       if accs is not None else None))

            # ---------------- transformer layers ----------------
            for l in range(NLAYERS):
                AB = l * PL
                fw_sb = load_fw(l)       # lands during attention
                bor_l = load_bor(l)

                probe("xt", xT[0])
                # V token-major first: copies drain on gpsimd during Q/K proj
                for rt in range(8):
                    ps = Q.tile([128, 512], F32,
                                tag=("C" if rt % 2 == 0 else "B"), bufs=2,
                                name=f"v_{nc.next_id()}")
                    for kt in range(DT):
                        MM(ps, xT[kt][:, rt * 128:(rt + 1) * 128],
                           aw_sb[:, AW_V + kt * 512:AW_V + kt * 512 + 512],
                           start=(kt == 0), stop=(kt == DT - 1))
                    v3o = vv[rt].rearrange("p (h e) -> p h e", e=DK + 1)
                    nc.scalar.activation(
                        out=v3o[:, :, 0:DK],
                        in_=ps.rearrange("p (h d) -> p h d", d=DK),
                        func=AF.Copy)
                probe("v", vv[0])

                qT, qT_tag = alloc_act()
                proj_v(qT, AW_Q, AB + 0, xT)
                probe("q", qT[0])
                kT, kT_tag = alloc_act()
                proj_v(kT, AW_K, AB + 4, xT)
                probe("k", kT[0])
                gT, gT_tag = alloc_act()
                ctxT, ctx_tag = alloc_act()

                # ---------- interleaved attention ----------
                # units u = (b, hp, jp); ctx sub-chunks lag 2 units.
                pr_tiles = {}

                def do_scores(b, hp, jp):
                    psAs = []
                    for h01 in range(2):
                        psAs.append(Q.tile(
                            [128, 1024], F32, tag="A", bufs=2,
                            name=f"sc_{nc.next_id()}"))
                    for j2 in range(2):
                        jt = jp * 2 + j2
                        for h01 in range(2):
                            hs = slice(h01 * 64, h01 * 64 + 64)
                            MM(psAs[h01][:, j2 * 512:(j2 + 1) * 512],
                               kT[hp][hs, b * 512 + jt * 128:
                                      b * 512 + jt * 128 + 128],
                               qT[hp][hs, b * 512:(b + 1) * 512],
                               start=True, stop=True)
                    prs = []
                    for h01 in range(2):
                        pr = P.tile([128, 1024], BFE, tag="pr", bufs=8,
                                    name=f"pr_{nc.next_id()}")
                        nc.scalar.activation(out=pr, in_=psAs[h01],
                                             func=AF.Exp)
                        nc.vector.tensor_mul(
                            pr, pr,
                            posE_sb[:, jp * 1024:(jp + 1) * 1024])
                        probe("pr", pr)
                        prs.append(pr)
                    pr_tiles[(b, hp, jp)] = prs

                def do_gchunk(g):
                    nt, rc = g % DT, g // DT
                    ps = Q.tile([128, 512], F32, tag="C", bufs=2,
                                name=f"g_{nc.next_id()}")
                    for kt in range(DT):
                        MM(ps,
                           aw_sb[:, AW_G + kt * 512 + nt * 128:
                                 AW_G + kt * 512 + nt * 128 + 128],
                           xT[kt][:, rc * 512:(rc + 1) * 512],
                           start=(kt == 0), stop=(kt == DT - 1))
                    nc.scalar.activation(
                        out=gT[nt][:, rc * 512:(rc + 1) * 512], in_=ps,
                        func=AF.Tanh, bias=col(AB + 12 + nt))

                def do_ctx(k):
                    b = k // 8
                    hp = (k % 8) // 2
                    h01 = k % 2
                    h = hp * 2 + h01
                    pc = Q.tile([128, 512], F32, tag="B", bufs=2,
                                name=f"pc_{nc.next_id()}")
                    for jt in range(4):
                        MM(pc[0:DK + 1, :],
                           vv[b * 4 + jt][:, h * (DK + 1):
                                          (h + 1) * (DK + 1)],
                           pr_tiles[(b, hp, jt // 2)][h01]
                           [:, (jt % 2) * 512:(jt % 2) * 512 + 512],
                           start=(jt == 0), stop=(jt == 3))
                    probe("pc", pc[0:DK + 1, :])
                    dcp = P.tile([1, 512], F32R, tag="rden",
                                 bufs=2, name=f"dc_{nc.next_id()}")
                    nc.scalar.activation(out=dcp, in_=pc[DK:DK + 1, :],
                                         func=AF.Copy)
                    dnb = Q.tile([64, 512], F32, tag="C", bufs=2,
                                 name=f"bc_{nc.next_id()}")
                    MM(dnb, ones_row_r[:, 0:64], dcp,
                       start=True, stop=True)
                    pbc = P.tile([64, 512], F32, tag="rden", bufs=2,
                                 name=f"rb_{nc.next_id()}")
                    nc.vector.reciprocal_approx_fast(out=pbc, in_=dnb)
                    nc.vector.tensor_mul(
                        ctxT[hp][h01 * 64:h01 * 64 + 64,
                                 b * 512:(b + 1) * 512],
                        pc[0:DK, :], pbc)

                gmap = {0: 0, 1: 1, 2: 2, 4: 3}
                for u in range(16):
                    b, hp, jp = u // 8, (u % 8) // 2, u % 2
                    do_scores(b, hp, jp)
                    if u in gmap:
                        do_gchunk(gmap[u])
                    if u >= 2:
                        do_ctx(u - 2)
                do_gchunk(4)
                do_ctx(14)
                do_gchunk(5)
                do_ctx(15)
                do_gchunk(6)
                do_gchunk(7)
                free_act(qT_tag)
                free_act(kT_tag)

                probe("ctx", ctxT[0])
                probe("g", gT[0])

                # table switch to Sqrt while O-proj runs on PE
                warm_table(AF.Sqrt)

                # ---------- O projection + gated residual ----------
                # attP psum gets bo' added via K=1 ones matmul, then
                # res = x + (gT + 1) * attP   (0.5 gate factor in Wo/bo')
                res = alloc_res()
                for nt in range(DT):
                    for rc in range(2):
                        sl = slice(rc * 512, (rc + 1) * 512)
                        ps = Q.tile([128, 512], F32,
                                    tag=("B" if (nt * 2 + rc) % 2 == 0
                                         else "C"), bufs=2,
                                    name=f"o_{nc.next_id()}")
                        for kt in range(DT):
                            MM(ps,
                               aw_sb[:, AW_O + kt * 512 + nt * 128:
                                     AW_O + kt * 512 + nt * 128 + 128],
                               ctxT[kt][:, sl],
                               start=(kt == 0), stop=False)
                        MM(ps, bor_l[:, nt * 128:(nt + 1) * 128],
                           ones_512h, start=False, stop=True)
                        tm = P.tile([128, 512], BF, tag="scr", bufs=3,
                                    name=f"tm_{nc.next_id()}")
                        nc.vector.scalar_tensor_tensor(
                            tm, gT[nt][:, sl], 1.0, ps,
                            op0=OP.add, op1=OP.mult)
                        nc.vector.tensor_add(res[nt][:, sl], tm,
                                             xT[nt][:, sl])
                probe("att", res[0])
                free_act(xT_tag)
                free_act(gT_tag)
                free_act(ctx_tag)

                probe("res", res[0])
                x1, x1_tag = alloc_act()
                layernorm(res, AB + 16, AB + 20, x1)
                warm_table(AF.Sigmoid, x1[0])
                probe("x1", x1[0])

                # prefetch next layer's attention weights
                if l + 1 < NLAYERS:
                    aw_next = load_aw(l + 1)

                # ---------------- FF ----------------
                res2 = alloc_res()
                for rc in range(2):
                    sl = slice(rc * 512, (rc + 1) * 512)
                    accA = [Q.tile([128, 1024], F32, tag="A", bufs=2,
                                   name=f"fa_{nc.next_id()}")
                            for _ in range(2)]
                    accs = [accA[0][:, 0:512], accA[0][:, 512:1024],
                            accA[1][:, 0:512], accA[1][:, 512:1024]]
                    for nt in range(FT):
                        pg = Q.tile([128, 512], F32, tag="C", bufs=2,
                                    name=f"pg_{nc.next_id()}")
                        for kt in range(DT):
                            MM(pg,
                               fw_sb[:, FW_G + kt * 2048 + nt * 128:
                                     FW_G + kt * 2048 + nt * 128 + 128],
                               x1[kt][:, sl],
                               start=(kt == 0), stop=(kt == DT - 1))
                        tg = P.tile([128, 512], BF, tag="fsc", bufs=3,
                                    name=f"tg_{nc.next_id()}")
                        nc.scalar.activation(out=tg, in_=pg, func=AF.Sigmoid,
                                             bias=col(AB + 48 + nt))
                        p1 = Q.tile([128, 512], F32, tag="B", bufs=2,
                                    name=f"p1_{nc.next_id()}")
                        for kt in range(DT):
                            MM(p1,
                               fw_sb[:, FW_1 + kt * 2048 + nt * 128:
                                     FW_1 + kt * 2048 + nt * 128 + 128],
                               x1[kt][:, sl],
                               start=(kt == 0), stop=(kt == DT - 1))
                        f = P.tile([128, 512], BF, tag="fsc", bufs=3,
                                   name=f"f_{nc.next_id()}")
                        nc.vector.scalar_tensor_tensor(
                            f, p1, col(AB + 32 + nt), tg,
                            op0=OP.add, op1=OP.mult)
                        for dt_ in range(DT):
                            MM(accs[dt_],
                               fw_sb[:, FW_2 + nt * 512 + dt_ * 128:
                                     FW_2 + nt * 512 + dt_ * 128 + 128],
                               f, start=(nt == 0), stop=(nt == FT - 1))
                    for dt_ in range(DT):
                        nc.vector.scalar_tensor_tensor(
                            res2[dt_][:, sl], accs[dt_], col(AB + 64 + dt_),
                            x1[dt_][:, sl], op0=OP.add, op1=OP.add)
                probe("res2", res2[0])
                free_act(x1_tag)

                warm_table(AF.Sqrt, res2[0])
                xT, xT_tag = alloc_act()
                if l == NLAYERS - 1:
                    xsums = [P.tile([128, BL], F32, tag="hacc", bufs=4,
                                    name=f"xs_{nc.next_id()}")
                             for _ in range(DT)]
                    # head weights + xo-projection are independent of
                    # xsums: DMAs overlap LN2, matmuls fill its chain gap
                    hw1t, hw1_tag = alloc_act()
                    nc.sync.dma_start(out=hw1t[0], in_=hw1_d[:, 0:1024])
                    nc.sync.dma_start(out=hw1t[1], in_=hw1_d[:, 1024:2048])
                    ow_sb = hw1t[2][0:FO, 0:D]
                    nc.sync.dma_start(out=ow_sb, in_=ow_d[:, :])
                    hT_xo = []
                    for nt in range(DT):
                        ps = Q.tile([128, BL], F32, tag="B", bufs=2,
                                    name=f"ho_{nc.next_id()}")
                        MM(ps, ow_sb[:, nt * 128:(nt + 1) * 128], xo_sb,
                           start=True, stop=True)
                        ht = P.tile([128, BL], BF, tag="hT", bufs=8,
                                    name=f"hx_{nc.next_id()}")
                        nc.vector.tensor_scalar(out=ht, in0=ps,
                                                scalar1=col(HB + 4 + nt),
                                                scalar2=None, op0=OP.add)
                        hT_xo.append(ht)
                else:
                    xsums = None
                layernorm(res2, AB + 24, AB + 28, xT, accs=xsums)
                probe("xout", xT[0])
                warm_table(AF.Exp, xT[0])
                if l + 1 < NLAYERS:
                    aw_sb = aw_next

            # ---------------- head ----------------
            hT = []
            for kt in range(DT):
                ht = P.tile([128, BL], BF, tag="hT", bufs=8,
                            name=f"hm_{nc.next_id()}")
                nc.vector.tensor_scalar(out=ht, in0=xsums[kt],
                                        scalar1=1.0 / S,
                                        scalar2=None, op0=OP.mult)
                hT.append(ht)
            hT.extend(hT_xo)

            eps1 = eps2[0:1, :]
            warm_table(AF.Sqrt, hT[0])

            def head_ln_relu(zt, n_tiles, nfeat, cs, cb, outtag):
                s1p = Q.tile([1, BL], F32, tag="B", bufs=2,
                             name=f"hs1_{nc.next_id()}")
                for kt in range(n_tiles):
                    MM(s1p, ones_col, zt[kt], start=(kt == 0),
                       stop=(kt == n_tiles - 1))
                s2p = Q.tile([1, BL], F32, tag="C", bufs=2,
                             name=f"hs2_{nc.next_id()}")
                for kt in range(n_tiles):
                    z2 = P.tile([128, BL], BF, tag="hd2", bufs=4,
                                name=f"z2_{nc.next_id()}")
                    nc.vector.tensor_mul(z2, zt[kt], zt[kt])
                    MM(s2p, ones_col, z2, start=(kt == 0),
                       stop=(kt == n_tiles - 1))
                mu = P.tile([1, BL], F32R, tag="hmu", bufs=4,
                            name=f"hmu_{nc.next_id()}")
                nc.vector.tensor_scalar(out=mu, in0=s1p,
                                        scalar1=1.0 / nfeat,
                                        scalar2=None, op0=OP.mult)
                m2 = P.tile([1, BL], F32, tag="hln", bufs=8,
                            name=f"hm2_{nc.next_id()}")
                nc.vector.tensor_scalar(out=m2, in0=s2p,
                                        scalar1=1.0 / nfeat,
                                        scalar2=None, op0=OP.mult)
                var = P.tile([1, BL], F32, tag="hln", bufs=8,
                             name=f"hva_{nc.next_id()}")
                nc.vector.scalar_tensor_tensor(
                    var, mu, -1.0, mu, op0=OP.mult, op1=OP.mult)
                nc.vector.tensor_add(var, var, m2)
                sq = P.tile([1, BL], F32, tag="hln", bufs=8,
                            name=f"hsq_{nc.next_id()}")
                nc.scalar.activation(out=sq, in_=var, func=AF.Sqrt,
                                     bias=eps1)
                rs = P.tile([1, BL], F32, tag="hmu", bufs=4,
                            name=f"hrs_{nc.next_id()}")
                nc.vector.reciprocal_approx_fast(out=rs, in_=sq)
                rsr = P.tile([1, BL], F32R, tag="hmu", bufs=4,
                             name=f"hrr_{nc.next_id()}")
                nc.vector.tensor_copy(rsr, rs)
                mub = Q.tile([128, BL], F32, tag="C", bufs=2,
                             name=f"hmb_{nc.next_id()}")
                MM(mub, ones_row_r, mu, start=True, stop=True)
                rsb = Q.tile([128, BL], F32, tag="B", bufs=2,
                             name=f"hrb_{nc.next_id()}")
                MM(rsb, ones_row_r, rsr, start=True, stop=True)
                outs = []
                for kt in range(n_tiles):
                    t1 = P.tile([128, BL], F32, tag="hd", bufs=8,
                                name=f"ht1_{nc.next_id()}")
                    nc.vector.tensor_tensor(t1, zt[kt], mub, OP.subtract)
                    t2 = P.tile([128, BL], F32, tag="hd", bufs=8,
                                name=f"ht2_{nc.next_id()}")
                    nc.vector.scalar_tensor_tensor(
                        t2, t1, col(cs + kt), rsb, op0=OP.mult, op1=OP.mult)
                    o = P.tile([128, BL], BF, tag=outtag, bufs=4,
                               name=f"ho_{nc.next_id()}")
                    nc.scalar.activation(out=o, in_=t2, func=AF.Relu,
                                         bias=col(cb + kt))
                    outs.append(o)
                return outs

            # fc1 [1024 -> 256]  (weights in recycled act tiles)
            def hw1_sl(c0, c1):
                t = hw1t[c0 // 1024]
                return t[:, c0 % 1024:c0 % 1024 + (c1 - c0)]

            z1 = []
            for nt in range(2):
                ps = Q.tile([128, BL], F32, tag="B", bufs=2,
                            name=f"f1_{nc.next_id()}")
                for kt in range(8):
                    c0 = kt * 256 + nt * 128
                    MM(ps, hw1_sl(c0, c0 + 128), hT[kt],
                       start=(kt == 0), stop=(kt == 7))
                z = P.tile([128, BL], BF, tag="z1", bufs=2,
                           name=f"z1_{nc.next_id()}")
                nc.vector.tensor_scalar(out=z, in0=ps,
                                        scalar1=col(HB + 8 + nt),
                                        scalar2=None, op0=OP.add)
                z1.append(z)
            h1 = head_ln_relu(z1, 2, 256, HB + 10, HB + 12, "h1")

            # fc2 [256 -> 128]
            ps = Q.tile([128, BL], F32, tag="B", bufs=2,
                        name=f"f2_{nc.next_id()}")
            for kt in range(2):
                MM(ps, hw2_sb[:, kt * 128:(kt + 1) * 128], h1[kt],
                   start=(kt == 0), stop=(kt == 1))
            z2_ = P.tile([128, BL], BF, tag="z2", bufs=2,
                         name=f"z2h_{nc.next_id()}")
            nc.vector.tensor_scalar(out=z2_, in0=ps, scalar1=col(HB + 14),
                                    scalar2=None, op0=OP.add)
            h2 = head_ln_relu([z2_], 1, 128, HB + 15, HB + 16, "h2")

            # fc3 [128 -> 1]
            ps = Q.tile([1, BL], F32, tag="C", bufs=2,
                        name=f"f3_{nc.next_id()}")
            MM(ps, hw2_sb[:, 256:257], h2[0], start=True, stop=True)
            out_sb = P.tile([1, BL], F32, tag="outsb", bufs=1)
            nc.vector.tensor_scalar(out=out_sb, in0=ps,
                                    scalar1=pb_sb[0:1, HB + 18:HB + 19],
                                    scalar2=None, op0=OP.add)
            nc.sync.dma_start(out=out_ext[:, :], in_=out_sb)

    nc.compile()
    return nc


def _tile_w(W):
    """[K*128, Dout] -> [128, K*Dout] bf16 (kt-major blocks)."""
    K = W.shape[0] // 128
    return np.ascontiguousarray(
        W.reshape(K, 128, -1).transpose(1, 0, 2).reshape(128, -1)
    ).astype(NPBF)


def _cols(pb, base, vec):
    """Pack vec[len=128*n] into pb[:, base:base+n] column-major."""
    v = np.asarray(vec, np.float32).reshape(-1, 128).T
    pb[:, base:base + v.shape[1]] = v


def _pack_shared(inputs):
    f32 = np.float32
    g = lambda k: np.asarray(inputs[k], f32)

    aw = np.zeros((L, 128, AW_COLS), NPBF)
    fw = np.zeros((L, 128, FW_COLS), NPBF)
    pb = np.zeros((128, NP), f32)
    bor = np.zeros((L, D), f32)
    Wq, bq = g("Wq"), g("bq")
    Wk, bk = g("Wk"), g("bk")
    Wv, bv = g("Wv"), g("bv")
    Wo, bo = g("Wo"), g("bo")
    Wg, bg = g("Wg"), g("bg")
    Wf1, bf1 = g("Wf1"), g("bf1")
    Wfg, bfg = g("Wfg"), g("bfg")
    Wf2, bf2 = g("Wf2"), g("bf2")
    for l in range(L):
        aw[l][:, AW_Q:AW_K] = _tile_w(Wq[l] * 0.125)
        aw[l][:, AW_K:AW_V] = _tile_w(Wk[l])
        aw[l][:, AW_V:AW_O] = _tile_w(Wv[l])
        # 0.5: sigmoid gate = (tanh(z/2)+1)/2, folded into Wo/bo'
        aw[l][:, AW_O:AW_G] = _tile_w(Wo[l] * 0.5)
        aw[l][:, AW_G:] = _tile_w(Wg[l] * 0.5)
        fw[l][:, FW_1:FW_G] = _tile_w(Wf1[l])
        fw[l][:, FW_G:FW_2] = _tile_w(Wfg[l])
        fw[l][:, FW_2:] = _tile_w(Wf2[l])
        AB = l * PL
        _cols(pb, AB + 0, bq[l] * 0.125)
        _cols(pb, AB + 4, bk[l])
        _cols(pb, AB + 12, bg[l] * 0.5)
        _cols(pb, AB + 16, g("ln1_s")[l])
        _cols(pb, AB + 20, g("ln1_b")[l])
        _cols(pb, AB + 24, g("ln2_s")[l])
        _cols(pb, AB + 28, g("ln2_b")[l])
        _cols(pb, AB + 32, bf1[l])
        _cols(pb, AB + 48, bfg[l])
        _cols(pb, AB + 64, bf2[l])
        # bo' = (bo + bv @ Wo) * 0.5, added into the O-proj PSUM by a
        # K=1 matmul (stationary = bor row, moving = ones)
        bor[l] = (bo[l] + bv[l] @ Wo[l]) * 0.5
    _cols(pb, HB + 0, g("cgm_b"))
    _cols(pb, HB + 4, g("other_b"))
    _cols(pb, HB + 8, g("fb1"))
    _cols(pb, HB + 10, g("fln1_s"))
    _cols(pb, HB + 12, g("fln1_b"))
    pb[:, HB + 14] = g("fb2")
    pb[:, HB + 15] = g("fln2_s")
    pb[:, HB + 16] = g("fln2_b")
    pb[:, HB + 17] = g("fW3")[:, 0]
    pb[0, HB + 18] = g("fb3")[0]

    # posE: exp(pos_bias) in scores-transposed layout
    rbar = g("rel_emb").mean(axis=1)            # [1023]
    posE = np.zeros((128, 2048), f32)
    Jv = np.arange(128)[:, None]
    Iv = np.arange(512)[None, :]
    for jt in range(4):
        idx = 511 - 128 * jt - Jv + Iv
        posE[:, jt * 512:(jt + 1) * 512] = np.exp(rbar[idx])

    return {
        "cgmW": g("cgm_W").astype(NPBF),
        "posE": posE.astype(NPBFE),
        "aw": aw, "fw": fw, "pb": pb, "bor": bor.astype(NPBF),
        "hw1": _tile_w(g("fW1")),
        "hw2": np.concatenate([_tile_w(g("fW2")),
                               g("fW3").astype(NPBF)], axis=1),
        "ow": g("other_W").astype(NPBF),
    }


def _get_nc():
    if "nc" not in _CACHE:
        _CACHE["nc"] = _build()
    return _CACHE["nc"]


def kernel(**inputs):
    shared = _pack_shared(inputs)
    x_cgm = np.asarray(inputs["x_cgm"], np.float32)
    x_other = np.asarray(inputs["x_other"], np.float32)
    in_maps = []
    for c in range(NCORES):
        m = dict(shared)
        xs = x_cgm[c * BL:(c + 1) * BL].reshape(R, FC).T
        m["xin"] = np.ascontiguousarray(xs).astype(NPBF)
        m["xo"] = np.ascontiguousarray(
            x_other[c * BL:(c + 1) * BL].T).astype(NPBF)
        in_maps.append(m)

    nc = _get_nc()
    trace = bool(int(os.environ.get("KTRACE", "0")))
    res = run_bass_kernel_spmd(nc, in_maps, core_ids=list(range(NCORES)),
                               trace=trace)
    _CACHE["last_res"] = res
    out = np.concatenate(
        [res.results[c]["out"].reshape(BL, 1) for c in range(NCORES)], axis=0)
    return out.astype(np.float32)
